# revision 3
# baseline (speedup 1.0000x reference)
"""Trainium2 Bass kernel v4 for nn_HSR_2_25116968747549 (gnn_message_passing).

Math: degenerate self-edge graph => the network collapses to
    t   = prelu(x @ M1 + v1, 0.01)
    a   = rsqrt(var_row(t) + eps)        (LayerNorm fold; mean removal and
                                          gamma folded into M2c host-side)
    out = prelu(a * (t @ M2c) + v2, 0.01)

Implementation highlights (evolved v1->v4 by trace analysis):
 * bf16 matmuls (fp32 matmuls run LOW_HIGH double-pass: ~4x slower).
 * Phase A flipped: host sends xT [64,1024]; 1 weight load + 2 N=512
   matmuls; ACT hardware Prelu (alpha=0.01, per-partition v1 bias) -> bf16.
 * t and t^2 stacked in one [128,1024] SBUF tile; ONE matmul per 128-row
   tile computes z=t@M2c, s1=sum(t), s2=sum(t^2) (cols 64/65)  -- no
   transposes, no separate stats matmuls, row-major output.
 * Prelu (parametric_relu) shares the ACT table with Sqrt -> no mid-kernel
   ACT table reloads (Lrelu does NOT: it lives in a sqrt-less table).
 * ONE input DMA + ONE weight DMA (v1 bias bitcast-packed into the bf16
   weight tensor): DMA completion semaphores lag ~2us and serialize near
   global queue drain, so fewer DMAs => earlier compute start.
 * Per-half stats (split PSUM tiles keep dependency tracking fine-grained;
   it is tensor-granular) + hardware Abs_reciprocal_sqrt for the rstd.
 * Wide broadcast finish: stride-0 broadcast APs let one tensor_tensor +
   one scalar_tensor_tensor handle 4 tiles at once.
 * Output is partition-major [128, 512] (host permutes back): one 2KB DMA
   descriptor per partition instead of eight 256B ones.
 * Output halves on two different HWDGE queues (Sync + ACT).
"""

import numpy as np

B, W, D, H = 256, 32, 64, 4
N = B * W
NCORES = 8
RPC = N // NCORES          # rows per core = 1024
TILES = RPC // 128         # 8 tiles of 128 rows
EPS = 1e-5
SLOPE = 0.01
def _fold_weights(inp):
    f = lambda k: np.asarray(inp[k], np.float64)
    M1 = f("Wl1") @ f("linw1") @ f("w1")
    v1 = (f("bl1") + f("cb1")) @ f("linw1") @ f("w1") + f("b1")
    A2w = f("Wl2") @ f("linw2") @ f("w2")
    M2 = f("gamma")[:, None] * A2w
    v2 = f("beta") @ A2w + (f("bl2") + f("cb2")) @ f("linw2") @ f("w2") + f("b2")
    Cm = np.eye(D) - 1.0 / D
    M2c = Cm @ M2
    return M1, v1, M2c, v2


def _edges_degenerate(src, dst):
    src = np.asarray(src)
    dst = np.asarray(dst)
    return src.shape == dst.shape and np.array_equal(src, dst) and np.all(
        np.bincount(dst.astype(np.int64), minlength=N)[:N] > 0
    )


def _numpy_fallback(inp):
    x = np.asarray(inp["x"], np.float32).reshape(N, D)
    src = np.asarray(inp["edge_src"]).astype(np.int64)
    dst = np.asarray(inp["edge_dst"]).astype(np.int64)

    def gat(xf, Wl, bl, Wr, br, att, cb, linw):
        xl = (xf @ Wl + bl).reshape(N, H, D)
        xr = (xf @ Wr + br).reshape(N, H, D)
        e = xl[src] + xr[dst]
        e = np.where(e > 0, e, 0.2 * e)
        logits = np.einsum("ehd,hd->eh", e, att)
        m = np.full((N, H), -np.inf, np.float32)
        np.maximum.at(m, dst, logits)
        ex = np.exp(logits - m[dst])
        den = np.zeros((N, H), np.float32)
        np.add.at(den, dst, ex)
        alpha = ex / den[dst]
        out = np.zeros((N, H, D), np.float32)
        np.add.at(out, dst, xl[src] * alpha[:, :, None])
        return (out.reshape(N, H * D) + cb) @ linw

    g = lambda k: np.asarray(inp[k], np.float32)
    lr = lambda t, a: np.where(t > 0, t, a * t)
    out = gat(x, g("Wl1"), g("bl1"), g("Wr1"), g("br1"), g("att1"), g("cb1"), g("linw1"))
    out = lr(out @ g("w1") + g("b1"), 0.01)
    mu = out.mean(-1, keepdims=True)
    var = ((out - mu) ** 2).mean(-1, keepdims=True)
    out = (out - mu) / np.sqrt(var + EPS) * g("gamma") + g("beta")
    out = gat(out, g("Wl2"), g("bl2"), g("Wr2"), g("br2"), g("att2"), g("cb2"), g("linw2"))
    out = lr(out @ g("w2") + g("b2"), 0.01)
    return out.reshape(B, W, D).astype(np.float32)


def build_bass():
    from concourse import bacc, mybir
    import concourse.tile as tile

    fp32 = mybir.dt.float32
    bf16 = mybir.dt.bfloat16
    Act = mybir.ActivationFunctionType
    Alu = mybir.AluOpType

    nc = bacc.Bacc()
    # wt layout (bf16 [128, 196]):
    #   [0:64, 0:64]   = M1                      (phase-A lhsT)
    #   [:, 64:130]    = stacked phase-C rhs:
    #        rows 0:64:   [M2c | ones | 0]
    #        rows 64:128: [ 0  |  0   | ones]
    #   [0, 130:194]   = v2                      (broadcast outer-product row)
    #   [0:64, 194:196] = v1 as fp32 bytes       (bitcast ACT bias column)
    xt_d = nc.declare_dram_parameter("xt", [D, RPC], bf16, isOutput=False)
    wt_d = nc.declare_dram_parameter("wt", [D, 196], bf16, isOutput=False)
    # partition-major output: each SBUF partition writes one contiguous
    # 2KB DRAM row -> 128 big DMA descriptors instead of 1024 small ones
    # (descriptor generation was ~1.4us for the 256B-chunk layout).
    y_d = nc.declare_dram_parameter("y", [128, TILES * D], fp32, isOutput=True)

    with tile.TileContext(nc) as tc:
        with (
            tc.tile_pool(name="const", bufs=1) as cpool,
            tc.tile_pool(name="psum", bufs=1, space="PSUM") as ppool,
        ):
            xt = cpool.tile([D, RPC], bf16, tag="xt")
            wt = cpool.tile([128, 196], bf16, tag="wt")
            tsq = cpool.tile([128, RPC], bf16, tag="tsq")
            onesb = cpool.tile([1, 128], bf16, tag="onesb")
            epsb = cpool.tile([128, 1], fp32, tag="epsb")
            v2b = cpool.tile([128, D], fp32, tag="v2b")
            stats = cpool.tile([128, 5 * TILES], fp32, tag="stats")
            u_sb = cpool.tile([128, TILES * D], fp32, tag="u_sb")
            o_sb = cpool.tile([128, TILES, D], fp32, tag="o_sb")
            warm = cpool.tile([1, 1], fp32, tag="warm")

            # split PSUM tiles: dependency tracking is tensor-granular, so
            # separate tiles per pipeline half keep deps fine-grained
            pA0 = ppool.tile([D, 512], fp32, tag="pA0")            # bank 0
            pA1 = ppool.tile([D, 512], fp32, tag="pA1")            # bank 1
            pC0 = ppool.tile([128, 2, 2, 256], fp32, tag="pC0")    # banks 2-3
            pC1 = ppool.tile([128, 2, 2, 256], fp32, tag="pC1")    # banks 4-5
            pV = ppool.tile([128, D], fp32, tag="pV")              # bank 6
            pCs = (pC0, pC1)
            pAs = (pA0, pA1)

            # constants + ACT table warm-up (Prelu and Sqrt share a table)
            nc.vector.memset(onesb[:], 1.0)
            nc.vector.memset(epsb[:], EPS)
            nc.vector.memset(warm[:], 1.0)
            nc.scalar.activation(out=warm[:], in_=warm[:], func=Act.Prelu,
                                 alpha=SLOPE)

            # input halves on Sync queue; weights on ACT queue -- only rows
            # 0:64 come from DRAM, rows 64:128 of the stacked block are
            # constants built by memset
            nc.vector.memset(wt[D:128, 64:130], 0.0)
            nc.vector.memset(wt[D:128, 129:130], 1.0 / D)
            nc.sync.dma_start(out=xt[:, 0:512], in_=xt_d[:, 0:512])
            nc.sync.dma_start(out=xt[:, 512:1024], in_=xt_d[:, 512:1024])
            nc.scalar.dma_start(out=wt[0:D, :], in_=wt_d[:])


            # v2 broadcast tile: ones(128) (x) v2  ->  [128, 64]
            nc.tensor.matmul(out=pV[:], lhsT=onesb[:],
                             rhs=wt[0:1, 130:194], start=True, stop=True)
            nc.vector.tensor_copy(out=v2b[:], in_=pV[:])

            # phase A: tT = Prelu(M1^T xT + v1)
            wc_ap = wt[0:D, 194:196].bitcast(fp32)
            for h in range(2):
                sl = slice(512 * h, 512 * (h + 1))
                nc.tensor.matmul(
                    out=pAs[h][:], lhsT=wt[0:D, 0:D],
                    rhs=xt[:, sl], start=True, stop=True,
                )
                nc.scalar.activation(
                    out=tsq[0:D, sl], in_=pAs[h][:], func=Act.Prelu,
                    bias=wc_ap, scale=1.0, alpha=SLOPE,
                )
                nc.vector.tensor_tensor(
                    out=tsq[D:128, sl], in0=tsq[0:D, sl],
                    in1=tsq[0:D, sl], op=Alu.mult,
                )

            # phase C: one matmul per tile -> z | s1 | s2
            for i in range(TILES):
                nc.tensor.matmul(
                    out=pCs[i // 4][:, (i % 4) // 2, i % 2, 0:66],
                    lhsT=tsq[:, 128 * i:128 * (i + 1)],
                    rhs=wt[:, 64:130], start=True, stop=True,
                )

            # stats per half with per-half rsqrt: h0's finish can start
            # while h1's stats still run
            u = stats[:, 0:8]
            msq = stats[:, 8:16]
            var = stats[:, 16:24]
            a8 = stats[:, 32:40]
            yv = y_d[:]

            def stats_half(h):
                c4 = slice(4 * h, 4 * (h + 1))
                s1 = pCs[h][:, :, :, 64:65]
                s2 = pCs[h][:, :, :, 65:66]
                nc.vector.tensor_scalar(
                    out=u[:, c4], in0=s1, scalar1=1.0, scalar2=None,
                    op0=Alu.mult)
                nc.vector.tensor_tensor(
                    out=msq[:, c4], in0=u[:, c4], in1=u[:, c4], op=Alu.mult)
                nc.vector.scalar_tensor_tensor(
                    out=var[:, c4], in0=s2, scalar=1.0, in1=msq[:, c4],
                    op0=Alu.mult, op1=Alu.subtract)
                nc.scalar.activation(out=a8[:, c4], in_=var[:, c4],
                                     func=Act.Abs_reciprocal_sqrt,
                                     bias=epsb[:])

            def finish_half(h):
                c4 = slice(4 * h, 4 * (h + 1))
                usl = slice(256 * h, 256 * (h + 1))
                nc.vector.tensor_tensor(
                    out=u_sb[:, usl], in0=pCs[h][:, :, :, 0:64],
                    in1=a8[:, c4].unsqueeze(2).broadcast_to([128, 4, 64]),
                    op=Alu.mult)
                nc.vector.scalar_tensor_tensor(
                    out=u_sb[:, usl], in0=u_sb[:, usl], scalar=1.0,
                    in1=v2b[:].unsqueeze(1).broadcast_to([128, 4, 64]),
                    op0=Alu.mult, op1=Alu.add)
                nc.scalar.activation(
                    out=o_sb[:, c4, :], in_=u_sb[:, usl],
                    func=Act.Prelu, alpha=SLOPE)
                if h == 0:
                    nc.sync.dma_start(out=yv[:, 0:256], in_=o_sb[:, 0:4, :])
                else:
                    nc.scalar.dma_start(out=yv[:, 256:512], in_=o_sb[:, 4:8, :])

            stats_half(0)
            stats_half(1)
            finish_half(0)
            finish_half(1)

    return nc


def _prep_inputs(inp):
    import ml_dtypes
    M1, v1, M2c, v2 = _fold_weights(inp)
    wt = np.zeros((D, 196), np.float32)
    wt[0:D, 0:64] = M1
    wt[0:D, 64:128] = M2c
    wt[0:D, 128] = 1.0 / D
    wt[0, 130:194] = v2
    wt = wt.astype(ml_dtypes.bfloat16)
    wt[0:D, 194:196] = (
        v1.astype(np.float32).reshape(D, 1).view(ml_dtypes.bfloat16))

    xf = np.asarray(inp["x"], np.float32).reshape(N, D)
    in_maps = []
    for c in range(NCORES):
        xs = xf[c * RPC:(c + 1) * RPC]
        xtc = np.ascontiguousarray(xs.T).astype(ml_dtypes.bfloat16)
        in_maps.append({"xt": xtc, "wt": wt})
    return in_maps


def kernel(**inputs):
    if not _edges_degenerate(inputs["edge_src"], inputs["edge_dst"]):
        return _numpy_fallback(inputs)

    from concourse.bass_utils import run_bass_kernel_spmd

    in_maps = _prep_inputs(inputs)
    nc = build_bass()
    if not nc.is_finalized():
        nc.finalize()
    res = run_bass_kernel_spmd(nc, in_maps, list(range(NCORES)))
    global LAST_RESULT
    LAST_RESULT = res
    outs = []
    for r in res.results:
        yc = r["y"].reshape(128, TILES, D).transpose(1, 0, 2).reshape(RPC, D)
        outs.append(yc)
    out = np.concatenate(outs, 0)
    return out.reshape(B, W, D).astype(np.float32)


LAST_RESULT = None


if __name__ == "__main__":
    print("kernel v4 module ok")


# revision 4
# speedup vs baseline: 1.0833x; 1.0833x over previous
"""Trainium2 Bass kernel for nn_HSR_2_25116968747549 (gnn_message_passing).

The reference's edge construction (tile(B,1).reshape(2,-1), an index-mixing
bug preserved from torch) makes edge_src == edge_dst for every edge: all
edges are self-loops, so each GATv2 layer's segment-softmax scatter is the
identity on xl and the network collapses (weights folded on host) to

    t   = prelu(x @ M1 + v1, 0.01)
    a   = rsqrt(var_row(t) + eps)        (LayerNorm fold; mean removal and
                                          gamma folded into M2c host-side)
    out = prelu(a * (t @ M2c) + v2, 0.01)

Sharding: data-parallel, core c owns rows [1024c, 1024(c+1)).

Implementation highlights (evolved by perfetto/NTFF trace analysis):
 * bf16 matmuls (fp32 matmuls run LOW_HIGH double-pass: ~4x slower).
 * Phase A flipped: host sends xT [64,1024]; 1 weight load + 2 N=512
   matmuls; ACT hardware Prelu (alpha=0.01, per-partition v1 bias) -> bf16.
 * t and t^2 stacked in one [128,1024] SBUF tile; ONE matmul per 128-row
   tile computes z=t@M2c, s1=mean(t), s2=mean(t^2) (cols 64/65, the 1/64
   folded into the weight columns) -- no transposes, no separate stats
   matmuls, row-major output.
 * Prelu (parametric_relu) shares its ACT table with Abs_reciprocal_sqrt ->
   one ACT table load total (Lrelu does NOT: it lives in a sqrt-less table).
 * Few DMAs (v1 bias bitcast-packed into the bf16 weight tensor; constant
   weight rows built by memset): DMA completion semaphores lag ~1-2us and
   fire near global queue drain, so fewer DMAs => earlier compute start.
 * Per-half stats + per-half rsqrt (split PSUM tiles keep the tile
   framework's tensor-granular dependency tracking fine-grained).
 * Wide broadcast finish: stride-0 broadcast APs let one tensor_tensor +
   one scalar_tensor_tensor handle 4 tiles at once.
 * Output is partition-major [128, 512] (host permutes back): one 2KB DMA
   descriptor per partition instead of eight 256B ones; halves go out on
   two different HWDGE queues (Sync + ACT).
"""

import numpy as np

B, W, D, H = 256, 32, 64, 4
N = B * W
NCORES = 8
RPC = N // NCORES          # rows per core = 1024
TILES = RPC // 128         # 8 tiles of 128 rows
EPS = 1e-5
SLOPE = 0.01


def _fold_weights(inp):
    f = lambda k: np.asarray(inp[k], np.float64)
    M1 = f("Wl1") @ f("linw1") @ f("w1")
    v1 = (f("bl1") + f("cb1")) @ f("linw1") @ f("w1") + f("b1")
    A2w = f("Wl2") @ f("linw2") @ f("w2")
    M2 = f("gamma")[:, None] * A2w
    v2 = f("beta") @ A2w + (f("bl2") + f("cb2")) @ f("linw2") @ f("w2") + f("b2")
    Cm = np.eye(D) - 1.0 / D
    M2c = Cm @ M2
    return M1, v1, M2c, v2


def _edges_degenerate(src, dst):
    src = np.asarray(src)
    dst = np.asarray(dst)
    return src.shape == dst.shape and np.array_equal(src, dst) and np.all(
        np.bincount(dst.astype(np.int64), minlength=N)[:N] > 0
    )


def _numpy_fallback(inp):
    x = np.asarray(inp["x"], np.float32).reshape(N, D)
    src = np.asarray(inp["edge_src"]).astype(np.int64)
    dst = np.asarray(inp["edge_dst"]).astype(np.int64)

    def gat(xf, Wl, bl, Wr, br, att, cb, linw):
        xl = (xf @ Wl + bl).reshape(N, H, D)
        xr = (xf @ Wr + br).reshape(N, H, D)
        e = xl[src] + xr[dst]
        e = np.where(e > 0, e, 0.2 * e)
        logits = np.einsum("ehd,hd->eh", e, att)
        m = np.full((N, H), -np.inf, np.float32)
        np.maximum.at(m, dst, logits)
        ex = np.exp(logits - m[dst])
        den = np.zeros((N, H), np.float32)
        np.add.at(den, dst, ex)
        alpha = ex / den[dst]
        out = np.zeros((N, H, D), np.float32)
        np.add.at(out, dst, xl[src] * alpha[:, :, None])
        return (out.reshape(N, H * D) + cb) @ linw

    g = lambda k: np.asarray(inp[k], np.float32)
    lr = lambda t, a: np.where(t > 0, t, a * t)
    out = gat(x, g("Wl1"), g("bl1"), g("Wr1"), g("br1"), g("att1"), g("cb1"), g("linw1"))
    out = lr(out @ g("w1") + g("b1"), 0.01)
    mu = out.mean(-1, keepdims=True)
    var = ((out - mu) ** 2).mean(-1, keepdims=True)
    out = (out - mu) / np.sqrt(var + EPS) * g("gamma") + g("beta")
    out = gat(out, g("Wl2"), g("bl2"), g("Wr2"), g("br2"), g("att2"), g("cb2"), g("linw2"))
    out = lr(out @ g("w2") + g("b2"), 0.01)
    return out.reshape(B, W, D).astype(np.float32)


def build_bass():
    from concourse import bacc, mybir
    import concourse.tile as tile

    fp32 = mybir.dt.float32
    bf16 = mybir.dt.bfloat16
    Act = mybir.ActivationFunctionType
    Alu = mybir.AluOpType

    nc = bacc.Bacc()
    # wt layout (bf16 [128, 196]):
    #   [0:64, 0:64]   = M1                      (phase-A lhsT)
    #   [:, 64:130]    = stacked phase-C rhs:
    #        rows 0:64:   [M2c | ones | 0]
    #        rows 64:128: [ 0  |  0   | ones]
    #   [0, 130:194]   = v2                      (broadcast outer-product row)
    #   [0:64, 194:196] = v1 as fp32 bytes       (bitcast ACT bias column)
    xt_d = nc.declare_dram_parameter("xt", [D, RPC], bf16, isOutput=False)
    wt_d = nc.declare_dram_parameter("wt", [D, 196], bf16, isOutput=False)
    # partition-major output: each SBUF partition writes one contiguous
    # 2KB DRAM row -> 128 big DMA descriptors instead of 1024 small ones
    # (descriptor generation was ~1.4us for the 256B-chunk layout).
    y_d = nc.declare_dram_parameter("y", [128, TILES * D], fp32, isOutput=True)

    with tile.TileContext(nc) as tc:
        with (
            tc.tile_pool(name="const", bufs=1) as cpool,
            tc.tile_pool(name="psum", bufs=1, space="PSUM") as ppool,
        ):
            xt = cpool.tile([D, RPC], bf16, tag="xt")
            wt = cpool.tile([128, 196], bf16, tag="wt")
            tsq = cpool.tile([128, RPC], bf16, tag="tsq")
            onesb = cpool.tile([1, 128], bf16, tag="onesb")
            epsb = cpool.tile([128, 1], fp32, tag="epsb")
            v2b = cpool.tile([128, D], fp32, tag="v2b")
            stats = cpool.tile([128, 5 * TILES], fp32, tag="stats")
            u_sb = cpool.tile([128, TILES * D], fp32, tag="u_sb")
            o_sb = cpool.tile([128, TILES, D], fp32, tag="o_sb")
            warm = cpool.tile([1, 1], fp32, tag="warm")

            # split PSUM tiles: dependency tracking is tensor-granular, so
            # separate tiles per pipeline half keep deps fine-grained
            pA0 = ppool.tile([D, 512], fp32, tag="pA0")            # bank 0
            pA1 = ppool.tile([D, 512], fp32, tag="pA1")            # bank 1
            pC0 = ppool.tile([128, 2, 2, 256], fp32, tag="pC0")    # banks 2-3
            pC1 = ppool.tile([128, 2, 2, 256], fp32, tag="pC1")    # banks 4-5
            pV = ppool.tile([128, D], fp32, tag="pV")              # bank 6
            pCs = (pC0, pC1)
            pAs = (pA0, pA1)

            # constants + ACT table warm-up (Prelu and Sqrt share a table)
            nc.vector.memset(onesb[:], 1.0)
            nc.vector.memset(epsb[:], EPS)
            nc.vector.memset(warm[:], 1.0)
            nc.scalar.activation(out=warm[:], in_=warm[:], func=Act.Prelu,
                                 alpha=SLOPE)

            # input halves on Sync queue; weights on ACT queue -- only rows
            # 0:64 come from DRAM, rows 64:128 of the stacked block are
            # constants built by memset
            nc.vector.memset(wt[D:128, 64:130], 0.0)
            nc.vector.memset(wt[D:128, 129:130], 1.0 / D)
            nc.sync.dma_start(out=xt[:, 0:512], in_=xt_d[:, 0:512])
            nc.sync.dma_start(out=xt[:, 512:1024], in_=xt_d[:, 512:1024])
            nc.scalar.dma_start(out=wt[0:D, :], in_=wt_d[:])


            # v2 broadcast tile: ones(128) (x) v2  ->  [128, 64]
            nc.tensor.matmul(out=pV[:], lhsT=onesb[:],
                             rhs=wt[0:1, 130:194], start=True, stop=True)
            nc.vector.tensor_copy(out=v2b[:], in_=pV[:])

            # phase A: tT = Prelu(M1^T xT + v1)
            wc_ap = wt[0:D, 194:196].bitcast(fp32)
            for h in range(2):
                sl = slice(512 * h, 512 * (h + 1))
                nc.tensor.matmul(
                    out=pAs[h][:], lhsT=wt[0:D, 0:D],
                    rhs=xt[:, sl], start=True, stop=True,
                )
                nc.scalar.activation(
                    out=tsq[0:D, sl], in_=pAs[h][:], func=Act.Prelu,
                    bias=wc_ap, scale=1.0, alpha=SLOPE,
                )
                nc.vector.tensor_tensor(
                    out=tsq[D:128, sl], in0=tsq[0:D, sl],
                    in1=tsq[0:D, sl], op=Alu.mult,
                )

            # phase C: one matmul per tile -> z | s1 | s2
            for i in range(TILES):
                nc.tensor.matmul(
                    out=pCs[i // 4][:, (i % 4) // 2, i % 2, 0:66],
                    lhsT=tsq[:, 128 * i:128 * (i + 1)],
                    rhs=wt[:, 64:130], start=True, stop=True,
                )

            # stats per half with per-half rsqrt: h0's finish can start
            # while h1's stats still run
            u = stats[:, 0:8]
            msq = stats[:, 8:16]
            var = stats[:, 16:24]
            a8 = stats[:, 32:40]
            yv = y_d[:]

            def stats_half(h):
                c4 = slice(4 * h, 4 * (h + 1))
                s1 = pCs[h][:, :, :, 64:65]
                s2 = pCs[h][:, :, :, 65:66]
                nc.vector.tensor_scalar(
                    out=u[:, c4], in0=s1, scalar1=1.0, scalar2=None,
                    op0=Alu.mult)
                nc.vector.tensor_tensor(
                    out=msq[:, c4], in0=u[:, c4], in1=u[:, c4], op=Alu.mult)
                nc.vector.scalar_tensor_tensor(
                    out=var[:, c4], in0=s2, scalar=1.0, in1=msq[:, c4],
                    op0=Alu.mult, op1=Alu.subtract)
                nc.scalar.activation(out=a8[:, c4], in_=var[:, c4],
                                     func=Act.Abs_reciprocal_sqrt,
                                     bias=epsb[:])

            def finish_half(h):
                c4 = slice(4 * h, 4 * (h + 1))
                usl = slice(256 * h, 256 * (h + 1))
                nc.vector.tensor_tensor(
                    out=u_sb[:, usl], in0=pCs[h][:, :, :, 0:64],
                    in1=a8[:, c4].unsqueeze(2).broadcast_to([128, 4, 64]),
                    op=Alu.mult)
                nc.vector.scalar_tensor_tensor(
                    out=u_sb[:, usl], in0=u_sb[:, usl], scalar=1.0,
                    in1=v2b[:].unsqueeze(1).broadcast_to([128, 4, 64]),
                    op0=Alu.mult, op1=Alu.add)
                nc.scalar.activation(
                    out=o_sb[:, c4, :], in_=u_sb[:, usl],
                    func=Act.Prelu, alpha=SLOPE)
                if h == 0:
                    nc.sync.dma_start(out=yv[:, 0:256], in_=o_sb[:, 0:4, :])
                else:
                    nc.scalar.dma_start(out=yv[:, 256:512], in_=o_sb[:, 4:8, :])

            stats_half(0)
            stats_half(1)
            finish_half(0)
            finish_half(1)

    return nc


def _prep_inputs(inp):
    import ml_dtypes
    M1, v1, M2c, v2 = _fold_weights(inp)
    wt = np.zeros((D, 196), np.float32)
    wt[0:D, 0:64] = M1
    wt[0:D, 64:128] = M2c
    wt[0:D, 128] = 1.0 / D
    wt[0, 130:194] = v2
    wt = wt.astype(ml_dtypes.bfloat16)
    wt[0:D, 194:196] = (
        v1.astype(np.float32).reshape(D, 1).view(ml_dtypes.bfloat16))

    xf = np.asarray(inp["x"], np.float32).reshape(N, D)
    in_maps = []
    for c in range(NCORES):
        xs = xf[c * RPC:(c + 1) * RPC]
        xtc = np.ascontiguousarray(xs.T).astype(ml_dtypes.bfloat16)
        in_maps.append({"xt": xtc, "wt": wt})
    return in_maps


def kernel(**inputs):
    if not _edges_degenerate(inputs["edge_src"], inputs["edge_dst"]):
        return _numpy_fallback(inputs)

    from concourse.bass_utils import run_bass_kernel_spmd

    in_maps = _prep_inputs(inputs)
    nc = build_bass()
    if not nc.is_finalized():
        nc.finalize()
    res = run_bass_kernel_spmd(nc, in_maps, list(range(NCORES)))
    global LAST_RESULT
    LAST_RESULT = res
    outs = []
    for r in res.results:
        yc = r["y"].reshape(128, TILES, D).transpose(1, 0, 2).reshape(RPC, D)
        outs.append(yc)
    out = np.concatenate(outs, 0)
    return out.reshape(B, W, D).astype(np.float32)


LAST_RESULT = None


if __name__ == "__main__":
    print("kernel v4 module ok")
